# revision 1
# baseline (speedup 1.0000x reference)
"""3-layer GAT (DGL GATConv, 1 head) on Trainium2, sharded over 8 NeuronCores.

Strategy (graph/data parallel by destination node):
  * Nodes split into 8 blocks of 6250; each core owns edges whose dst falls in
    its block (edges sorted by dst on host, packed into fixed-shape slots).
  * Per layer each core computes its block's node table rows
    ftl[n] = [ft(n) (F), el(n), 1.0, pad] with ft = h @ W, el = h @ (W @ al),
    plus er(n) = h @ (W @ ar) kept in a local per-block column.  Table shards
    are AllGathered so every core holds the full 50000-row table in DRAM.
  * Edge stage: supertile = 128 consecutive local dsts (= psum rows), NJ
    blocks of 128 edge slots (NJ = global max, pads contribute exactly 0).
    Per block one indirect DMA gathers the 130-col table rows for the edge
    sources (this deployment's walrus only supports one index per partition
    per indirect DMA, so the gather is the instruction-bound hot loop).
  * ex = exp(leaky_relu(el_src + er_dst)); er_dst is produced on chip:
    er row -> all-partitions broadcast via a rank-1 matmul, then a one-hot
    mask (iota == shift) reduce.  The same mask times ex forms the scatter
    matrix S, and matmuls accumulate psum[128 dsts, 130] += S_j^T @ X_j; the
    table's constant 1.0 column makes psum's last column the softmax
    denominator.  Softmax max-subtraction is skipped: logits are O(10) here,
    exp() cannot overflow fp32, and the result is mathematically identical.
  * Epilogue: out = relu(psum * 1/esum) written contiguously to the core's
    next-layer h block; layer 3 writes the external output shard.

Self-contained: hardcodes the problem shapes; host-side work is numpy only.
"""

import numpy as np

import concourse.bass as bass
import concourse.mybir as mybir
import concourse.tile as tile
from concourse import bacc
from concourse.bass import IndirectOffsetOnAxis
from concourse.bass_utils import run_bass_kernel_spmd
from concourse.masks import make_identity

FP = mybir.dt.float32
I32 = mybir.dt.int32

N_NODES = 50000
N_CORES = 8
IN_F = 128
NEG_SLOPE = 0.2
SUP = 128  # dsts per supertile (= psum rows)


class _Cfg:
    def __init__(self, n_nodes, n_cores, feats, nj):
        self.N = n_nodes
        self.C = n_cores
        self.NB = n_nodes // n_cores
        assert self.NB * n_cores == n_nodes
        self.feats = feats  # list of (F_in, F_out) per layer
        self.NSUP = -(-self.NB // SUP)
        self.NJ = nj  # edge blocks (of 128 slots) per supertile

    def table_width(self, li):
        # [ft (F), el, one, pad...] ; padded so rows are 16B aligned
        f = self.feats[li][1]
        return -(-(f + 2) // 4) * 4

    def gather_width(self, li):
        return self.feats[li][1] + 2  # ft, el, one

    def key(self):
        return (self.N, self.C, tuple(self.feats), self.NJ)


def _pack_edges(src, dst, cfg):
    """Sort by dst, shard by dst block, pack into [NSUP, 128, NJ] slots.

    Returns (NJ, src_idx i32 [C,NSUP,128,NJ], shift f32 same).
    Slot (s, k, j) holds the supertile's edge number j*128+k; pads get
    src=0 and shift=128 (matches no psum row, contributes exactly zero).
    """
    C, NB, NSUP = cfg.C, cfg.NB, cfg.NSUP
    order = np.argsort(dst, kind="stable")
    src_s = src[order].astype(np.int64)
    dst_s = dst[order].astype(np.int64)

    core_lo = np.searchsorted(dst_s, np.arange(C) * NB)
    core_hi = np.searchsorted(dst_s, (np.arange(C) + 1) * NB)

    nj = 0
    percore = []
    for c in range(C):
        lo, hi = core_lo[c], core_hi[c]
        dloc = dst_s[lo:hi] - c * NB
        sid = dloc // SUP
        sc = np.bincount(sid, minlength=NSUP)
        nj = max(nj, -(-int(sc.max()) // 128))
        percore.append((lo, hi, dloc, sid, sc))

    src_out = np.zeros((C, NSUP, 128, nj), np.int32)
    shift_out = np.full((C, NSUP, 128, nj), float(SUP), np.float32)
    slots_s = nj * 128
    for c in range(C):
        lo, hi, dloc, sid, sc = percore[c]
        sstart = np.concatenate([[0], np.cumsum(sc)[:-1]])
        pos = np.arange(hi - lo) - sstart[sid]
        slot = sid * slots_s + pos
        g_src = np.zeros(NSUP * slots_s, np.int64)
        g_shift = np.full(NSUP * slots_s, float(SUP), np.float32)
        g_src[slot] = src_s[lo:hi]
        g_shift[slot] = (dloc % SUP).astype(np.float32)
        # [NSUP, nj, 128] -> [NSUP, 128, nj]
        src_out[c] = g_src.reshape(NSUP, nj, 128).transpose(0, 2, 1)
        shift_out[c] = g_shift.reshape(NSUP, nj, 128).transpose(0, 2, 1)
    return nj, src_out, shift_out


def _build(cfg, has_bias):
    """Build + compile the (core-independent) Bass program."""
    nc = bacc.Bacc(
        "TRN2",
        target_bir_lowering=False,
        debug=False,
        num_devices=cfg.C,
    )
    NB, NSUP, NJ = cfg.NB, cfg.NSUP, cfg.NJ
    NL = len(cfg.feats)

    feat_c = nc.dram_tensor("feat_c", [NB, IN_F], FP, kind="ExternalInput")
    iota_in = nc.dram_tensor("iota", [128, SUP], FP, kind="ExternalInput")
    srcidx = nc.dram_tensor("srcidx", [NSUP, 128, NJ], I32, kind="ExternalInput")
    shiftv = nc.dram_tensor("shiftv", [NSUP, 128, NJ], FP, kind="ExternalInput")
    waug = [
        nc.dram_tensor(f"waug{li}", [cfg.feats[li][0], cfg.feats[li][1] + 2], FP,
                       kind="ExternalInput")
        for li in range(NL)
    ]
    bias_in = [
        nc.dram_tensor(f"bias{li}", [128, cfg.feats[li][1]], FP, kind="ExternalInput")
        if has_bias[li] else None
        for li in range(NL)
    ]

    tbl_shard = [
        nc.dram_tensor(f"tbl_shard{li}", [NB, cfg.table_width(li)], FP)
        for li in range(NL)
    ]
    shared_kw = {"addr_space": "Shared"} if cfg.C > 4 else {}
    tbl_full = [
        nc.dram_tensor(f"tbl_full{li}", [cfg.N, cfg.table_width(li)], FP,
                       **shared_kw)
        for li in range(NL)
    ]
    er_own = [
        nc.dram_tensor(f"er_own{li}", [NB, 1], FP)
        for li in range(NL)
    ]
    h_mid = [
        nc.dram_tensor(f"h_mid{li}", [NB, cfg.feats[li][1]], FP)
        for li in range(NL - 1)
    ]
    out_c = nc.dram_tensor("out_c", [NB, cfg.feats[-1][1]], FP, kind="ExternalOutput")

    n_row_tiles = -(-NB // 128)
    replica = [list(range(cfg.C))]

    with tile.TileContext(nc, num_cores=cfg.C) as tc:
        with (
            tc.tile_pool(name="const", bufs=1) as constp,
            tc.tile_pool(name="nodein", bufs=3) as nodein,
            tc.tile_pool(name="nodet", bufs=2) as nodet,
            tc.tile_pool(name="nodepsum", bufs=1, space="PSUM") as nodepsum,
            tc.tile_pool(name="stage", bufs=3) as stagep,
            tc.tile_pool(name="idx", bufs=4) as idxp,
            tc.tile_pool(name="xg", bufs=4) as xgp,
            tc.tile_pool(name="ex", bufs=2) as exp_,
            tc.tile_pool(name="sm", bufs=2) as smp,
            tc.tile_pool(name="epsum", bufs=3, space="PSUM") as epsum,
            tc.tile_pool(name="eout", bufs=3) as eoutp,
        ):
            ident = constp.tile([128, 128], FP, tag="ident")
            make_identity(nc, ident[:])
            iota_sb = constp.tile([128, SUP], FP, tag="iota")
            nc.sync.dma_start(out=iota_sb[:], in_=iota_in[:])
            ones_row = constp.tile([1, SUP], FP, tag="ones")
            nc.vector.memset(ones_row[:], 1.0)

            for li in range(NL):
                f_in, f_out = cfg.feats[li]
                tw = cfg.table_width(li)
                gw = cfg.gather_width(li)

                wsb = constp.tile([f_in, f_out + 2], FP, tag=f"waug{li}")
                nc.sync.dma_start(out=wsb[:], in_=waug[li][:])
                if has_bias[li]:
                    bsb = constp.tile([128, f_out], FP, tag=f"bias{li}")
                    nc.sync.dma_start(out=bsb[:], in_=bias_in[li][:])

                # ---- node stage: own block rows -> table shard + er column ----
                hsrc = feat_c if li == 0 else h_mid[li - 1]
                for t in range(n_row_tiles):
                    r0 = t * 128
                    rows = min(128, NB - r0)
                    h_t = nodein.tile([128, f_in], FP, tag="h")
                    nc.sync.dma_start(out=h_t[:rows], in_=hsrc[r0:r0 + rows, :])
                    ps_t = nodepsum.tile([f_in, 128], FP, tag="pT")
                    nc.tensor.transpose(out=ps_t[:, :rows], in_=h_t[:rows],
                                        identity=ident[:rows, :rows])
                    hT = nodet.tile([f_in, 128], FP, tag="hT")
                    nc.scalar.copy(out=hT[:, :rows], in_=ps_t[:, :rows])
                    ps2 = nodepsum.tile([128, f_out + 2], FP, tag="p2")
                    nc.tensor.matmul(out=ps2[:rows], lhsT=hT[:, :rows], rhs=wsb[:],
                                     start=True, stop=True)
                    st = stagep.tile([128, tw], FP, tag="st")
                    # cols [ft, el] then the constant-one column
                    nc.scalar.copy(out=st[:rows, 0:f_out + 1],
                                   in_=ps2[:rows, 0:f_out + 1])
                    nc.vector.memset(st[:rows, f_out + 1:f_out + 2], 1.0)
                    if tw > f_out + 2:
                        nc.vector.memset(st[:rows, f_out + 2:tw], 0.0)
                    nc.sync.dma_start(out=tbl_shard[li][r0:r0 + rows, :],
                                      in_=st[:rows])
                    er_st = stagep.tile([128, 1], FP, tag="er_st")
                    nc.scalar.copy(out=er_st[:rows],
                                   in_=ps2[:rows, f_out + 1:f_out + 2])
                    nc.sync.dma_start(out=er_own[li][r0:r0 + rows, :],
                                      in_=er_st[:rows])

                # ---- all-gather the node table ----
                nc.gpsimd.collective_compute(
                    "AllGather",
                    mybir.AluOpType.bypass,
                    replica_groups=replica,
                    ins=[tbl_shard[li][:]],
                    outs=[tbl_full[li][:]],
                )

                # ---- edge stage ----
                # gathers write disjoint slices of one tile, which serializes
                # per-tile; interleave gathers of GRP supertiles so GRP DMA
                # chains overlap.
                GRP = 3
                for s0 in range(0, NSUP, GRP):
                    grp = range(s0, min(s0 + GRP, NSUP))
                    src_g, x_g = {}, {}
                    for s in grp:
                        src_t = idxp.tile([128, NJ], I32, tag="src",
                                          name=f"srcg{li}_{s}")
                        nc.sync.dma_start(out=src_t[:], in_=srcidx[s])
                        src_g[s] = src_t
                        x_g[s] = xgp.tile([128, NJ, gw], FP, tag="x",
                                          name=f"xg{li}_{s}")
                    for j in range(NJ):
                        for s in grp:
                            nc.gpsimd.indirect_dma_start(
                                out=x_g[s][:, j, :],
                                out_offset=None,
                                in_=tbl_full[li][:],
                                in_offset=IndirectOffsetOnAxis(
                                    ap=src_g[s][:, j:j + 1], axis=0),
                            )
                    for s in grp:
                        r0 = s * SUP
                        rows = min(SUP, NB - r0)
                        x_t = x_g[s]
                        shf_t = idxp.tile([128, NJ], FP, tag="shf")
                        nc.sync.dma_start(out=shf_t[:], in_=shiftv[s])

                        # er for this supertile's dsts, broadcast to all partitions
                        er_sb = exp_.tile([1, SUP], FP, tag="er_row")
                        if rows < SUP:
                            nc.vector.memset(er_sb[:], 0.0)
                        nc.sync.dma_start(out=er_sb[:1, :rows],
                                          in_=er_own[li][r0:r0 + rows, 0][None, :])
                        eb_ps = nodepsum.tile([128, SUP], FP, tag="eb")
                        nc.tensor.matmul(out=eb_ps[:], lhsT=ones_row[:],
                                         rhs=er_sb[:], start=True, stop=True)

                        # one-hot mask M[k, j, w] = (iota[w] == shift[k, j])
                        m_t = smp.tile([128, NJ * SUP], FP, tag="m")
                        m3 = m_t[:].rearrange("p (j w) -> p j w", w=SUP)
                        nc.vector.tensor_tensor(
                            out=m3,
                            in0=iota_sb[:, None, :].to_broadcast([128, NJ, SUP]),
                            in1=shf_t[:, :, None].to_broadcast([128, NJ, SUP]),
                            op=mybir.AluOpType.is_equal,
                        )
                        # er_dst per edge = reduce_w(M * er_bcast)
                        tmp_t = smp.tile([128, NJ * SUP], FP, tag="tmp")
                        tmp3 = tmp_t[:].rearrange("p (j w) -> p j w", w=SUP)
                        nc.vector.tensor_tensor(
                            out=tmp3,
                            in0=m3,
                            in1=eb_ps[:, None, :].to_broadcast([128, NJ, SUP]),
                            op=mybir.AluOpType.mult,
                        )
                        er_e = exp_.tile([128, NJ], FP, tag="er_e")
                        nc.vector.tensor_reduce(
                            out=er_e[:], in_=tmp3,
                            axis=mybir.AxisListType.X, op=mybir.AluOpType.add)

                        # ex = exp(leaky_relu(el_src + er_dst))
                        e_t = exp_.tile([128, NJ], FP, tag="e")
                        nc.vector.tensor_tensor(out=e_t[:], in0=x_t[:, :, f_out],
                                                in1=er_e[:], op=mybir.AluOpType.add)
                        lr_t = exp_.tile([128, NJ], FP, tag="lr")
                        nc.vector.tensor_scalar_mul(out=lr_t[:], in0=e_t[:],
                                                    scalar1=NEG_SLOPE)
                        nc.vector.tensor_tensor(out=lr_t[:], in0=e_t[:], in1=lr_t[:],
                                                op=mybir.AluOpType.max)
                        ex_t = exp_.tile([128, NJ], FP, tag="exv")
                        nc.scalar.activation(out=ex_t[:], in_=lr_t[:],
                                             func=mybir.ActivationFunctionType.Exp)

                        # S = M * ex (in place on M)
                        nc.vector.tensor_tensor(
                            out=m3,
                            in0=m3,
                            in1=ex_t[:, :, None].to_broadcast([128, NJ, SUP]),
                            op=mybir.AluOpType.mult,
                        )

                        ps = epsum.tile([128, gw], FP, tag="eps")
                        for j in range(NJ):
                            nc.tensor.matmul(
                                out=ps[:],
                                lhsT=m_t[:, j * SUP:(j + 1) * SUP],
                                rhs=x_t[:, j, :],
                                start=(j == 0),
                                stop=(j == NJ - 1),
                            )

                        esum = eoutp.tile([128, 1], FP, tag="esum")
                        nc.vector.tensor_scalar_max(out=esum[:], in0=ps[:, gw - 1:gw],
                                                    scalar1=1e-30)
                        rec = eoutp.tile([128, 1], FP, tag="rec")
                        nc.vector.reciprocal(out=rec[:], in_=esum[:])

                        o_t = eoutp.tile([128, f_out], FP, tag="o")
                        if has_bias[li]:
                            nc.scalar.activation(out=o_t[:rows], in_=ps[:rows, 0:f_out],
                                                 func=mybir.ActivationFunctionType.Copy,
                                                 scale=rec[:rows, 0:1])
                            nc.vector.tensor_tensor(out=o_t[:rows], in0=o_t[:rows],
                                                    in1=bsb[:rows],
                                                    op=mybir.AluOpType.add)
                            nc.vector.tensor_scalar_max(out=o_t[:rows], in0=o_t[:rows],
                                                        scalar1=0.0)
                        else:
                            nc.scalar.activation(out=o_t[:rows], in_=ps[:rows, 0:f_out],
                                                 func=mybir.ActivationFunctionType.Relu,
                                                 scale=rec[:rows, 0:1])
                        dest = out_c if li == NL - 1 else h_mid[li]
                        nc.sync.dma_start(out=dest[r0:r0 + rows, :], in_=o_t[:rows])

    nc.compile()
    return nc


_CACHE = {}


def _get_program(cfg, has_bias):
    key = (cfg.key(), tuple(has_bias))
    if key not in _CACHE:
        _CACHE[key] = _build(cfg, has_bias)
    return _CACHE[key]


def _run(cfg, prep, **run_kwargs):
    nc = _get_program(cfg, prep["has_bias"])
    return run_bass_kernel_spmd(nc, prep["in_maps"], list(range(cfg.C)),
                                **run_kwargs)


def _prepare(feat, src, dst, Ws, als, ars, bs, cfg):
    nj, src_idx, shift = _pack_edges(np.asarray(src), np.asarray(dst), cfg)
    assert nj == cfg.NJ, f"packed NJ={nj} != cfg.NJ={cfg.NJ}"

    has_bias = tuple(bool(np.any(np.asarray(b) != 0)) for b in bs)
    iota = np.broadcast_to(
        np.arange(SUP, dtype=np.float32)[None, :], (128, SUP)).copy()

    feat = np.asarray(feat, np.float32)
    in_maps = []
    for c in range(cfg.C):
        m = {
            "feat_c": np.ascontiguousarray(feat[c * cfg.NB:(c + 1) * cfg.NB]),
            "iota": iota,
            "srcidx": src_idx[c],
            "shiftv": shift[c],
        }
        for li in range(len(cfg.feats)):
            W = np.asarray(Ws[li], np.float32)
            al = np.asarray(als[li], np.float32)
            ar = np.asarray(ars[li], np.float32)
            m[f"waug{li}"] = np.ascontiguousarray(
                np.concatenate([W, (W @ al)[:, None], (W @ ar)[:, None]], 1))
            if has_bias[li]:
                m[f"bias{li}"] = np.broadcast_to(
                    np.asarray(bs[li], np.float32)[None, :],
                    (128, cfg.feats[li][1])).copy()
        in_maps.append(m)
    return {"in_maps": in_maps, "has_bias": has_bias}


def kernel(feat, src, dst, W1, al1, ar1, b1, W2, al2, ar2, b2,
           W3, al3, ar3, b3):
    feats = [(128, 128), (128, 128), (128, 16)]
    src = np.asarray(src)
    dst = np.asarray(dst)
    probe = _Cfg(N_NODES, N_CORES, feats, 1)
    nj, _, _ = _pack_edges(src, dst, probe)
    cfg = _Cfg(N_NODES, N_CORES, feats, nj)
    prep = _prepare(feat, src, dst,
                    [W1, W2, W3], [al1, al2, al3], [ar1, ar2, ar3],
                    [b1, b2, b3], cfg)
    res = _run(cfg, prep).results
    out = np.concatenate(
        [res[c]["out_c"] for c in range(cfg.C)], 0).astype(np.float32)
    return out

